# revision 69
# baseline (speedup 1.0000x reference)
"""SAN Bottleneck (pairwise self-attention) Trainium2 kernel.

Sharding: 8 cores = 2 batches x 4 row-blocks of 14 rows (H=56). Each core
receives a reflect-padded input slice (20 rows x 62 cols), so the 7x7
unfold needs no runtime halo exchange and no edge special-casing.

Per-core pipeline (all batchnorms folded into per-channel scale/bias on host):
  bn1+relu -> x1/x2/x3 1x1 convs (matmuls)
  feat = relu(x1 - shifted-window(x2))            (fp16, DVE/GPSIMD)
  mm1 66->64 (fp16) -> relu+bias (ACT, PSUM evac)
  mm2 64->128 with 4x-replicated head weights -> exp+bias (ACT)
  softmax normalizer + aggregation: shifted-window products (DVE) and
  in-place pairwise tree adds over the 49 taps (DVE+GPSIMD)
  bn2+relu -> wc conv; the +bias and +identity ride the same PSUM via an
  identity-stationary matmul, so the output eviction is a single ACT op.

All constants ship in two packed blobs (one bf16, one f16) so startup is
4 DMAs instead of 25. Odd-parity window taps read x2/x3 directly at
offset+1 with stride-2 APs (no shifted copies). Input x ships as bf16.
"""

import numpy as np
import ml_dtypes

bf16_np = ml_dtypes.bfloat16

K = 7
PAD = 3
EPS = 1e-5
B, C, H, W = 2, 256, 56, 56
RB = 14              # rows per core
NQ = RB * W          # 784
ROWS = RB + 2 * PAD  # 20
WP = W + 2 * PAD     # 62
K2 = K * K
CHUNKS = [(0, 4), (4, 4), (8, 3), (11, 3)]

# bf16 blob column layout: w1(2x64) w2(2x64) w3(2x2x128) wc(2x2x128) I(128)
WB_W1 = 0
WB_W2 = 128
WB_W3 = 256
WB_WC = 768
WB_ID = 1280
WB_N = 1408
# f16 blob: cw1 (64 cols, rows 0:66), cw2 (128 cols, rows 0:128 doubled)
FB_CW1 = 0
FB_CW2 = 64
FB_N = 192

_BUILD_CACHE = {}


def _perm_channels():
    perm = np.zeros(256, np.int64)
    for t in range(2):
        for p in range(128):
            perm[t * 128 + p] = 8 * (p // 4) + 4 * t + (p % 4)
    return perm


def _build_program():
    if "nc" in _BUILD_CACHE:
        return _BUILD_CACHE["nc"]
    import concourse.bass as bass
    import concourse.bacc as bacc
    import concourse.tile as tile
    import concourse.mybir as mybir
    from contextlib import ExitStack

    f32 = mybir.dt.float32
    f16 = mybir.dt.float16
    bf16 = mybir.dt.bfloat16
    Alu = mybir.AluOpType
    Act = mybir.ActivationFunctionType

    nc = bacc.Bacc("TRN2", target_bir_lowering=False, num_devices=8)

    xp_d = nc.dram_tensor("xp", [128, 2, ROWS, WP], bf16, kind="ExternalInput")
    rsubp_d = nc.dram_tensor("rsubp", [2, K2, NQ], f16, kind="ExternalInput")
    wb_d = nc.dram_tensor("wb", [128, WB_N], bf16, kind="ExternalInput")
    fb_d = nc.dram_tensor("fb", [128, FB_N], f16, kind="ExternalInput")
    scal_d = nc.dram_tensor("scal", [128, 14], f32, kind="ExternalInput")
    y_d = nc.dram_tensor("y", [2, 128, RB, W], f32, kind="ExternalOutput")

    def win_ap(base, elem_off, ndj, nr):
        # [P][ndj dj (stride 2)][nr rows (stride WP)][56 cols] into a flat
        # padded [P, ROWS*WP] tile
        return bass.AP(
            tensor=base.tensor,
            offset=base.offset + elem_off,
            ap=[base.ap[0], [2, ndj], [WP, nr], [1, W]],
        )

    def kq_ap(base3, k0, ndj, nqc, nr):
        # [P][ndj (stride 2*nqc)][nr][56] into a [P, 49, nqc] tile at tap k0
        return bass.AP(
            tensor=base3.tensor,
            offset=base3.offset + k0 * nqc,
            ap=[base3.ap[0], [2 * nqc, ndj], [W, nr], [1, W]],
        )

    with tile.TileContext(nc) as tc, ExitStack() as stack:
        consts = stack.enter_context(tc.tile_pool(name="consts", bufs=1))
        xpp = stack.enter_context(tc.tile_pool(name="xpp", bufs=1))
        headsb = stack.enter_context(tc.tile_pool(name="headsb", bufs=1))

        wb = consts.tile([128, WB_N], bf16, tag="wb")
        fb = consts.tile([128, FB_N], f16, tag="fb")
        scals = consts.tile([128, 14], f32, tag="scals")
        dummy = consts.tile([128, 1], f32, tag="dummy")
        # warm the ACT function table off the critical path
        nc.vector.memset(dummy[:], 0.0)
        nc.scalar.activation(out=dummy[:], in_=dummy[:], func=Act.Exp)
        nc.sync.dma_start(out=scals[:], in_=scal_d[:])

        def w1v(kt):
            return wb[:, WB_W1 + 64 * kt:WB_W1 + 64 * (kt + 1)]

        def w2v(kt):
            return wb[:, WB_W2 + 64 * kt:WB_W2 + 64 * (kt + 1)]

        def w3v(kt, ot):
            o = WB_W3 + 256 * kt + 128 * ot
            return wb[:, o:o + 128]

        def wcv(kt, ot):
            o = WB_WC + 256 * kt + 128 * ot
            return wb[:, o:o + 128]

        idv = wb[:, WB_ID:WB_ID + 128]
        cw1s = fb[0:66, FB_CW1:FB_CW1 + 64]

        def cw2v(half):
            return fb[64 * half:64 * half + 64, FB_CW2:FB_CW2 + 128]

        a1 = [scals[:, 0:1], scals[:, 1:2]]
        b1f = [scals[:, 2:3], scals[:, 3:4]]
        b1p = scals[0:64, 4:5]
        b2p = scals[0:64, 5:6]
        b2f = scals[:, 6:7]
        cb2r = scals[:, 7:8]
        a3p = [scals[:, 8:9], scals[:, 9:10]]
        b3fp = [scals[:, 10:11], scals[:, 11:12]]
        bcb = [scals[:, 12:13], scals[:, 13:14]]

        xpt = xpp.tile([128, 2, ROWS, WP], bf16, tag="xpt")
        nc.sync.dma_start(out=xpt[:, 0], in_=xp_d[:, 0])
        nc.sync.dma_start(out=wb[:], in_=wb_d[:])
        nc.sync.dma_start(out=xpt[:, 1], in_=xp_d[:, 1])
        nc.sync.dma_start(out=fb[:], in_=fb_d[:])
        xps = [xpt[:, t, :, :] for t in range(2)]
        obn = [headsb.tile([128, ROWS * WP], bf16, tag=f"obn{t}", name=f"obn{t}") for t in range(2)]
        ccuts = [(0, 416), (416, 416), (832, 408)]
        for (o0, n) in ccuts:
            for t in range(2):
                nc.scalar.activation(
                    out=obn[t][:, o0:o0 + n],
                    in_=xps[t].rearrange("p r w -> p (r w)")[:, o0:o0 + n],
                    func=Act.Relu, bias=b1f[t], scale=a1[t])

        x1s = headsb.tile([64, RB, W], f16, tag="x1s")
        x2p = headsb.tile([64, ROWS * WP], f16, tag="x2p")
        x3ps = headsb.tile([128, 2, ROWS * WP], f16, tag="x3ps")

        with tc.tile_pool(name="pshead", bufs=4, space="PSUM") as pshead:
            for (o0, n) in ccuts:
                ps = pshead.tile([64, 416], f32, tag="ps64")
                for kt in range(2):
                    nc.tensor.matmul(
                        ps[:, :n], w2v(kt),
                        obn[kt][:, o0:o0 + n],
                        start=(kt == 0), stop=(kt == 1))
                nc.scalar.activation(out=x2p[:, o0:o0 + n], in_=ps[:, :n],
                                     func=Act.Identity, bias=b2p, scale=1.0)
            for half in range(2):
                ps = pshead.tile([64, 416], f32, tag="ps64")
                for kt in range(2):
                    rhs = obn[kt][:].rearrange("p (r w) -> p r w", w=WP)[
                        :, 3 + 7 * half:3 + 7 * (half + 1), 3:3 + W]
                    nc.tensor.matmul(ps[:, :392], w1v(kt),
                                     rhs,
                                     start=(kt == 0), stop=(kt == 1))
                nc.scalar.activation(
                    out=x1s[:, 7 * half:7 * (half + 1), :],
                    in_=ps[:, :392].rearrange("p (r w) -> p r w", w=W),
                    func=Act.Identity, bias=b1p, scale=1.0)
            for ot in range(2):
                for (o0, n) in ccuts:
                    ps = pshead.tile([128, 416], f32, tag="ps128")
                    for kt in range(2):
                        nc.tensor.matmul(
                            ps[:, :n], w3v(kt, ot),
                            obn[kt][:, o0:o0 + n],
                            start=(kt == 0), stop=(kt == 1))
                    nc.scalar.activation(out=x3ps[:, ot, o0:o0 + n],
                                         in_=ps[:, :n], func=Act.Copy)

        featp = stack.enter_context(tc.tile_pool(name="featp", bufs=2))
        h2p = stack.enter_context(tc.tile_pool(name="h2p", bufs=1))
        e4p = stack.enter_context(tc.tile_pool(name="e4p", bufs=2))
        prodp = stack.enter_context(tc.tile_pool(name="prodp", bufs=1))
        smallp = stack.enter_context(tc.tile_pool(name="smallp", bufs=2))
        # one PSUM tile holds the softmax normalizer and both aggregation
        # accumulators: [:,0:224]=Z, [:,256:480]=agg tile0, [:,512:736]=agg1
        pszp = stack.enter_context(tc.tile_pool(name="pszp", bufs=1, space="PSUM"))
        ps1p = stack.enter_context(tc.tile_pool(name="ps1p", bufs=2, space="PSUM"))
        ps2p = stack.enter_context(tc.tile_pool(name="ps2p", bufs=2, space="PSUM"))

        chunk_state = {}

        def tree_a(eng, t):
            # sum taps 0..17 into t[:,0,:]
            for (a, b, n) in [(0, 9, 9), (0, 4, 4), (0, 2, 2), (0, 1, 1),
                              (0, 8, 1)]:
                eng.tensor_tensor(out=t[:, a:a + n, :], in0=t[:, a:a + n, :],
                                  in1=t[:, b:b + n, :], op=Alu.add)

        def phase1(ci):
            (r0c, nr) = CHUNKS[ci]
            nqc = nr * W
            vc = K2 * nqc
            feat = featp.tile([66, K2, nqc], f16, tag="feat", name=f"feat{ci}")
            nc.sync.dma_start(out=feat[64:66, :, :],
                                in_=rsubp_d[:, :, r0c * W:r0c * W + nqc])

            fv = feat[0:64]
            x1v = x1s[:, r0c:r0c + nr, :]
            # window subtracts: dj{0,2,4,6} and dj5 on DVE, dj{1,3} on GPSIMD
            for di in range(K):
                for (eng, off, ndj) in ((nc.vector, 0, 4),
                                        (nc.vector, 5, 1),
                                        (nc.gpsimd, 1, 2)):
                    x2w = win_ap(x2p[:], (r0c + di) * WP + off, ndj, nr)
                    x1w = bass.AP(tensor=x1v.tensor, offset=x1v.offset,
                                  ap=[x1v.ap[0], [0, ndj], x1v.ap[1], x1v.ap[2]])
                    outw = kq_ap(fv, di * K + off, ndj, nqc, nr)
                    eng.tensor_tensor(out=outw, in0=x1w, in1=x2w,
                                      op=Alu.subtract)
            for bq in range(4):
                ks = (K2 * bq) // 4, (K2 * (bq + 1)) // 4
                nc.vector.tensor_scalar_max(
                    out=feat[0:64, ks[0]:ks[1], :].rearrange("p a b -> p (a b)"),
                    in0=feat[0:64, ks[0]:ks[1], :].rearrange("p a b -> p (a b)"),
                    scalar1=0.0)

            featf = feat[:].rearrange("p a b -> p (a b)")
            # mm1 packs two 512-col blocks onto the 128 PSUM partitions so
            # the relu eviction covers half the free size
            h2w = (vc // 1024) * 512 + min(512, vc % 1024)
            h2 = h2p.tile([128, h2w], f16, tag="h2")
            for j0 in range(0, vc, 1024):
                n = min(1024, vc - j0)
                ps1 = ps1p.tile([128, 512], f32, tag="ps1")
                n0 = min(512, n)
                nc.tensor.matmul(ps1[0:64, :n0], cw1s,
                                 featf[:, j0:j0 + n0],
                                 start=True, stop=True)
                if n > 512:
                    nc.tensor.matmul(ps1[64:128, :n - 512], cw1s,
                                     featf[:, j0 + 512:j0 + n],
                                     start=True, stop=True)
                    nc.scalar.activation(
                        out=h2[:, j0 // 2:j0 // 2 + n - 512],
                        in_=ps1[:, :n - 512],
                        func=Act.Relu, bias=b2f, scale=1.0)
                    if n - 512 < 512:
                        nc.scalar.activation(
                            out=h2[0:64, j0 // 2 + n - 512:j0 // 2 + 512],
                            in_=ps1[0:64, n - 512:512],
                            func=Act.Relu, bias=b2f[0:64], scale=1.0)
                else:
                    nc.scalar.activation(
                        out=h2[0:64, j0 // 2:j0 // 2 + n],
                        in_=ps1[0:64, :n],
                        func=Act.Relu, bias=b2f[0:64], scale=1.0)

            e4 = e4p.tile([128, K2, nqc], f16, tag="e4")
            e4f = e4[:].rearrange("p a b -> p (a b)")
            for j0 in range(0, vc, 1024):
                n = min(1024, vc - j0)
                ps2 = ps2p.tile([128, 1024], f32, tag="ps2")
                for s in range(0, n, 512):
                    sn = min(512, n - s)
                    half = (s // 512) % 2
                    nc.tensor.matmul(
                        ps2[:, s:s + sn], cw2v(half),
                        h2[64 * half:64 * half + 64,
                           j0 // 2 + s - 512 * half:j0 // 2 + s - 512 * half + sn],
                        start=True, stop=True)
                nc.scalar.activation(out=e4f[:, j0:j0 + n], in_=ps2[:, :n],
                                     func=Act.Exp, bias=cb2r, scale=1.0)

            chunk_state[ci] = (e4,)

        def phase2(ci):
            (r0c, nr) = CHUNKS[ci]
            nqc = nr * W
            (e4,) = chunk_state[ci]
            # softmax normalizer on PE: Z = sum_k e4[:,k,:] via 49
            # identity-stationary matmuls accumulating in PSUM
            psz = pszp.tile([128, 768], f32, tag="psz", name=f"psz{ci}")
            for k in range(K2):
                nc.tensor.matmul(psz[:, 0:nqc], idv, e4[:, k, :],
                                 start=(k == 0), stop=(k == K2 - 1))
            prods = []
            for ot in range(2):
                prodt = prodp.tile([128, K2, nqc], f16, tag=f"prod{ot}",
                                   name=f"prod{ot}")
                prods.append(prodt)
                for di in range(K):
                    for par in range(2):
                        ndj = 4 if par == 0 else 3
                        sv = x3ps[:, ot, :]
                        k0 = di * K + par
                        x3w = bass.AP(
                            tensor=sv.tensor,
                            offset=sv.offset + (r0c + di) * WP + par,
                            ap=[sv.ap[0], [2, ndj], [WP, nr], [1, W]])
                        e4w = kq_ap(e4[:], k0, ndj, nqc, nr)
                        outw = kq_ap(prods[ot][:], k0, ndj, nqc, nr)
                        nc.vector.tensor_tensor(out=outw, in0=e4w, in1=x3w,
                                                op=Alu.mult)
            # taps 12..48 summed on PE into the agg accumulators; taps 0..11
            # tree-summed on GPSIMD
            for ot in range(2):
                a0 = 256 + 256 * ot
                for k in range(18, K2):
                    nc.tensor.matmul(psz[:, a0:a0 + nqc], idv,
                                     prods[ot][:, k, :],
                                     start=(k == 18), stop=(k == K2 - 1))
                tree_a(nc.gpsimd, prods[ot])

            rz = smallp.tile([128, nqc], f32, tag="rz")
            nc.vector.reciprocal(out=rz[:], in_=psz[:, 0:nqc])

            outb = []
            for ot in range(2):
                a0 = 256 + 256 * ot
                ob = smallp.tile([128, nqc], f32, tag=f"ob{ot}", name=f"ob{ot}")
                ob2 = smallp.tile([128, nqc], bf16, tag=f"ob2{ot}", name=f"ob2{ot}")
                outb.append(ob2)
                nc.vector.tensor_tensor(out=ob[:], in0=psz[:, a0:a0 + nqc],
                                        in1=prods[ot][:, 0, :], op=Alu.add)
                nc.vector.scalar_tensor_tensor(
                    out=ob[:], in0=ob[:], scalar=1.0, in1=rz[:],
                    op0=Alu.mult, op1=Alu.mult)
                nc.scalar.activation(out=ob2[:], in_=ob[:], func=Act.Relu,
                                     bias=b3fp[ot], scale=a3p[ot])

            for oo in range(2):
                psw = ps2p.tile([128, 1024], f32, tag="ps2")
                for kt in range(2):
                    nc.tensor.matmul(psw[:, :nqc], wcv(kt, oo),
                                     outb[kt][:],
                                     start=(kt == 0), stop=False)
                xi = xps[oo][:, PAD + r0c:PAD + r0c + nr, PAD:PAD + W]
                nc.tensor.matmul(psw[:, :nqc], idv,
                                 xi, start=False, stop=True)
                ysb = smallp.tile([128, nqc], f32, tag=f"ysb{oo}", name=f"ysb{oo}")
                nc.scalar.activation(out=ysb[:], in_=psw[:, :nqc],
                                     func=Act.Identity, bias=bcb[oo], scale=1.0)
                nc.sync.dma_start(
                    out=y_d[oo][:, r0c:r0c + nr, :],
                    in_=ysb[:].rearrange("p (r w) -> p r w", w=W))

        phase1(0)
        for ci in range(1, len(CHUNKS)):
            phase1(ci)
            phase2(ci - 1)
        phase2(len(CHUNKS) - 1)

    nc.compile()
    _BUILD_CACHE["nc"] = nc
    return nc


def _host_prep(inputs):
    f = {k: np.asarray(v, np.float32) for k, v in inputs.items()}

    def fold(n):
        a = f[n + "_g"] / np.sqrt(f[n + "_rv"] + EPS)
        return a, f[n + "_b"] - f[n + "_rm"] * a

    a1, b1f = fold("bn1")
    ac, bc1 = fold("cwbn1")
    a2, b2f = fold("cwbn2")
    a3, b3f = fold("bn2")

    W1p = ac[:64, None] * f["w1"]
    b1p = ac[:64] * f["b1"] + bc1[:64]
    W2p = ac[:64, None] * f["w2"]
    b2p = ac[:64] * f["b2"]
    cw1p = a2[:, None] * f["cw1"]

    perm = _perm_channels()
    w3p = f["w3"][perm]
    a3p = a3[perm]
    b3fp = b3f[perm]
    rep = np.arange(128) // 4
    cw2r = f["cw2"][rep]
    cb2r = f["cb2"][rep]

    locw = np.tile(np.linspace(-1.0, 1.0, W, dtype=np.float32)[None, :], (H, 1))
    loch = np.tile(np.linspace(-1.0, 1.0, H, dtype=np.float32)[:, None], (1, W))
    loc = np.stack([locw, loch], 0)
    p = np.einsum("chw,oc->ohw", loc, f["pw"]) + f["pb"][:, None, None]
    pp = np.pad(p, ((0, 0), (PAD, PAD), (PAD, PAD)), mode="reflect")
    pu = np.stack([pp[:, i:i + H, j:j + W] for i in range(K) for j in range(K)], 1)
    subp = p[:, None] - pu
    rsubp = np.maximum(ac[64:66, None, None, None] * subp
                       + bc1[64:66, None, None, None], 0).astype(np.float16)

    xpad = np.pad(f["x"], ((0, 0), (0, 0), (PAD, PAD), (PAD, PAD)), mode="reflect")

    w1T = np.ascontiguousarray(W1p.T).reshape(2, 128, 64)
    w2T = np.ascontiguousarray(W2p.T).reshape(2, 128, 64)
    wc_perm = f["wc"][:, perm]

    wb = np.zeros((128, WB_N), np.float32)
    for kt in range(2):
        wb[:, WB_W1 + 64 * kt:WB_W1 + 64 * (kt + 1)] = w1T[kt]
        wb[:, WB_W2 + 64 * kt:WB_W2 + 64 * (kt + 1)] = w2T[kt]
        for ot in range(2):
            wb[:, WB_W3 + 256 * kt + 128 * ot:WB_W3 + 256 * kt + 128 * (ot + 1)] = \
                w3p[ot * 128:(ot + 1) * 128, kt * 128:(kt + 1) * 128].T
            wb[:, WB_WC + 256 * kt + 128 * ot:WB_WC + 256 * kt + 128 * (ot + 1)] = \
                wc_perm[ot * 128:(ot + 1) * 128, kt * 128:(kt + 1) * 128].T
    wb[:, WB_ID:WB_ID + 128] = np.eye(128, dtype=np.float32)

    fbb = np.zeros((128, FB_N), np.float32)
    fbb[0:66, FB_CW1:FB_CW1 + 64] = cw1p.T
    fbb[0:64, FB_CW2:FB_CW2 + 128] = cw2r.T
    fbb[64:128, FB_CW2:FB_CW2 + 128] = cw2r.T

    scal = np.zeros((128, 14), np.float32)
    scal[:, 0] = a1[:128]; scal[:, 1] = a1[128:]
    scal[:, 2] = b1f[:128]; scal[:, 3] = b1f[128:]
    scal[:64, 4] = b1p; scal[:64, 5] = b2p
    scal[:64, 6] = b2f; scal[64:, 6] = b2f
    scal[:, 7] = cb2r
    scal[:, 8] = a3p[:128]; scal[:, 9] = a3p[128:]
    scal[:, 10] = b3fp[:128]; scal[:, 11] = b3fp[128:]
    scal[:, 12] = f["bc"][:128]; scal[:, 13] = f["bc"][128:]

    shared = dict(wb=wb.astype(bf16_np), fb=fbb.astype(np.float16), scal=scal)
    in_maps = []
    for core in range(8):
        b, i = divmod(core, 4)
        r0 = RB * i
        m = dict(shared)
        xc = xpad[b].reshape(2, 128, H + 2 * PAD, WP)[:, :, r0:r0 + ROWS, :]
        m["xp"] = np.ascontiguousarray(
            xc.transpose(1, 0, 2, 3)).astype(bf16_np)
        m["rsubp"] = np.ascontiguousarray(
            rsubp[:, :, r0:r0 + RB, :].reshape(2, K2, NQ))
        in_maps.append(m)
    return in_maps


def kernel(**inputs):
    from concourse.bass_utils import run_bass_kernel_spmd
    nc = _build_program()
    in_maps = _host_prep(inputs)
    res = run_bass_kernel_spmd(nc, in_maps, core_ids=list(range(8)))
    global LAST_RESULTS
    LAST_RESULTS = res
    y = np.zeros((B, C, H, W), np.float32)
    for core in range(8):
        b, i = divmod(core, 4)
        yc = res.results[core]["y"]
        y[b, :, RB * i:RB * (i + 1), :] = yc.reshape(C, RB, W)
    return y


# revision 70
# speedup vs baseline: 1.0097x; 1.0097x over previous
"""SAN Bottleneck (pairwise self-attention) Trainium2 kernel.

Sharding: 8 cores = 2 batches x 4 row-blocks of 14 rows (H=56). Each core
receives a reflect-padded input slice (20 rows x 62 cols), so the 7x7
unfold needs no runtime halo exchange and no edge special-casing.

Per-core pipeline (all batchnorms folded into per-channel scale/bias on host):
  bn1+relu -> x1/x2/x3 1x1 convs (matmuls)
  feat = relu(x1 - shifted-window(x2))            (fp16, DVE/GPSIMD)
  mm1 66->64 (fp16) -> relu+bias (ACT, PSUM evac)
  mm2 64->128 with 4x-replicated head weights -> exp+bias (ACT)
  softmax normalizer + aggregation: shifted-window products (DVE) and
  in-place pairwise tree adds over the 49 taps (DVE+GPSIMD)
  bn2+relu -> wc conv; the +bias and +identity ride the same PSUM via an
  identity-stationary matmul, so the output eviction is a single ACT op.

All constants ship in two packed blobs (one bf16, one f16) so startup is
4 DMAs instead of 25. Odd-parity window taps read x2/x3 directly at
offset+1 with stride-2 APs (no shifted copies). Input x ships as bf16.
"""

import numpy as np
import ml_dtypes

bf16_np = ml_dtypes.bfloat16

K = 7
PAD = 3
EPS = 1e-5
B, C, H, W = 2, 256, 56, 56
RB = 14              # rows per core
NQ = RB * W          # 784
ROWS = RB + 2 * PAD  # 20
WP = W + 2 * PAD     # 62
K2 = K * K
CHUNKS = [(0, 4), (4, 4), (8, 3), (11, 3)]

# bf16 blob column layout: w1(2x64) w2(2x64) w3(2x2x128) wc(2x2x128) I(128)
WB_W1 = 0
WB_W2 = 128
WB_W3 = 256
WB_WC = 768
WB_ID = 1280
WB_N = 1408
# f16 blob: cw1 (64 cols, rows 0:66), cw2 (128 cols, rows 0:128 doubled)
FB_CW1 = 0
FB_CW2 = 64
FB_N = 192

_BUILD_CACHE = {}


def _perm_channels():
    perm = np.zeros(256, np.int64)
    for t in range(2):
        for p in range(128):
            perm[t * 128 + p] = 8 * (p // 4) + 4 * t + (p % 4)
    return perm


def _build_program():
    if "nc" in _BUILD_CACHE:
        return _BUILD_CACHE["nc"]
    import concourse.bass as bass
    import concourse.bacc as bacc
    import concourse.tile as tile
    import concourse.mybir as mybir
    from contextlib import ExitStack

    f32 = mybir.dt.float32
    f16 = mybir.dt.float16
    bf16 = mybir.dt.bfloat16
    Alu = mybir.AluOpType
    Act = mybir.ActivationFunctionType

    nc = bacc.Bacc("TRN2", target_bir_lowering=False, num_devices=8)

    xp_d = nc.dram_tensor("xp", [128, 2, ROWS, WP], bf16, kind="ExternalInput")
    rsubp_d = nc.dram_tensor("rsubp", [2, K2, NQ], f16, kind="ExternalInput")
    wb_d = nc.dram_tensor("wb", [128, WB_N], bf16, kind="ExternalInput")
    fb_d = nc.dram_tensor("fb", [128, FB_N], f16, kind="ExternalInput")
    scal_d = nc.dram_tensor("scal", [128, 14], f32, kind="ExternalInput")
    y_d = nc.dram_tensor("y", [2, 128, RB, W], f32, kind="ExternalOutput")

    def win_ap(base, elem_off, ndj, nr):
        # [P][ndj dj (stride 2)][nr rows (stride WP)][56 cols] into a flat
        # padded [P, ROWS*WP] tile
        return bass.AP(
            tensor=base.tensor,
            offset=base.offset + elem_off,
            ap=[base.ap[0], [2, ndj], [WP, nr], [1, W]],
        )

    def kq_ap(base3, k0, ndj, nqc, nr):
        # [P][ndj (stride 2*nqc)][nr][56] into a [P, 49, nqc] tile at tap k0
        return bass.AP(
            tensor=base3.tensor,
            offset=base3.offset + k0 * nqc,
            ap=[base3.ap[0], [2 * nqc, ndj], [W, nr], [1, W]],
        )

    with tile.TileContext(nc) as tc, ExitStack() as stack:
        consts = stack.enter_context(tc.tile_pool(name="consts", bufs=1))
        xpp = stack.enter_context(tc.tile_pool(name="xpp", bufs=1))
        headsb = stack.enter_context(tc.tile_pool(name="headsb", bufs=1))

        wb = consts.tile([128, WB_N], bf16, tag="wb")
        fb = consts.tile([128, FB_N], f16, tag="fb")
        scals = consts.tile([128, 14], f32, tag="scals")
        dummy = consts.tile([128, 1], f32, tag="dummy")
        # warm the ACT function table off the critical path
        nc.vector.memset(dummy[:], 0.0)
        nc.scalar.activation(out=dummy[:], in_=dummy[:], func=Act.Exp)
        nc.sync.dma_start(out=scals[:], in_=scal_d[:])

        def w1v(kt):
            return wb[:, WB_W1 + 64 * kt:WB_W1 + 64 * (kt + 1)]

        def w2v(kt):
            return wb[:, WB_W2 + 64 * kt:WB_W2 + 64 * (kt + 1)]

        def w3v(kt, ot):
            o = WB_W3 + 256 * kt + 128 * ot
            return wb[:, o:o + 128]

        def wcv(kt, ot):
            o = WB_WC + 256 * kt + 128 * ot
            return wb[:, o:o + 128]

        idv = wb[:, WB_ID:WB_ID + 128]
        cw1s = fb[0:66, FB_CW1:FB_CW1 + 64]

        def cw2v(half):
            return fb[64 * half:64 * half + 64, FB_CW2:FB_CW2 + 128]

        a1 = [scals[:, 0:1], scals[:, 1:2]]
        b1f = [scals[:, 2:3], scals[:, 3:4]]
        b1p = scals[0:64, 4:5]
        b2p = scals[0:64, 5:6]
        b2f = scals[:, 6:7]
        cb2r = scals[:, 7:8]
        a3p = [scals[:, 8:9], scals[:, 9:10]]
        b3fp = [scals[:, 10:11], scals[:, 11:12]]
        bcb = [scals[:, 12:13], scals[:, 13:14]]

        xpt = xpp.tile([128, 2, ROWS, WP], bf16, tag="xpt")
        nc.sync.dma_start(out=xpt[:, 0], in_=xp_d[:, 0])
        nc.sync.dma_start(out=wb[:], in_=wb_d[:])
        nc.sync.dma_start(out=xpt[:, 1], in_=xp_d[:, 1])
        nc.sync.dma_start(out=fb[:], in_=fb_d[:])
        xps = [xpt[:, t, :, :] for t in range(2)]
        obn = [headsb.tile([128, ROWS * WP], bf16, tag=f"obn{t}", name=f"obn{t}") for t in range(2)]
        ccuts = [(0, 416), (416, 416), (832, 408)]
        for (o0, n) in ccuts:
            for t in range(2):
                nc.scalar.activation(
                    out=obn[t][:, o0:o0 + n],
                    in_=xps[t].rearrange("p r w -> p (r w)")[:, o0:o0 + n],
                    func=Act.Relu, bias=b1f[t], scale=a1[t])

        x1s = headsb.tile([64, RB, W], f16, tag="x1s")
        x2p = headsb.tile([64, ROWS * WP], f16, tag="x2p")
        x3ps = headsb.tile([128, 2, ROWS * WP], f16, tag="x3ps")

        with tc.tile_pool(name="pshead", bufs=4, space="PSUM") as pshead:
            for (o0, n) in ccuts:
                ps = pshead.tile([64, 416], f32, tag="ps64")
                for kt in range(2):
                    nc.tensor.matmul(
                        ps[:, :n], w2v(kt),
                        obn[kt][:, o0:o0 + n],
                        start=(kt == 0), stop=(kt == 1))
                nc.scalar.activation(out=x2p[:, o0:o0 + n], in_=ps[:, :n],
                                     func=Act.Identity, bias=b2p, scale=1.0)
            for half in range(2):
                ps = pshead.tile([64, 416], f32, tag="ps64")
                for kt in range(2):
                    rhs = obn[kt][:].rearrange("p (r w) -> p r w", w=WP)[
                        :, 3 + 7 * half:3 + 7 * (half + 1), 3:3 + W]
                    nc.tensor.matmul(ps[:, :392], w1v(kt),
                                     rhs,
                                     start=(kt == 0), stop=(kt == 1))
                nc.scalar.activation(
                    out=x1s[:, 7 * half:7 * (half + 1), :],
                    in_=ps[:, :392].rearrange("p (r w) -> p r w", w=W),
                    func=Act.Identity, bias=b1p, scale=1.0)
            for ot in range(2):
                for (o0, n) in ccuts:
                    ps = pshead.tile([128, 416], f32, tag="ps128")
                    for kt in range(2):
                        nc.tensor.matmul(
                            ps[:, :n], w3v(kt, ot),
                            obn[kt][:, o0:o0 + n],
                            start=(kt == 0), stop=(kt == 1))
                    nc.scalar.activation(out=x3ps[:, ot, o0:o0 + n],
                                         in_=ps[:, :n], func=Act.Copy)

        featp = stack.enter_context(tc.tile_pool(name="featp", bufs=2))
        h2p = stack.enter_context(tc.tile_pool(name="h2p", bufs=1))
        e4p = stack.enter_context(tc.tile_pool(name="e4p", bufs=2))
        prodp = stack.enter_context(tc.tile_pool(name="prodp", bufs=1))
        smallp = stack.enter_context(tc.tile_pool(name="smallp", bufs=2))
        # one PSUM tile holds the softmax normalizer and both aggregation
        # accumulators: [:,0:224]=Z, [:,256:480]=agg tile0, [:,512:736]=agg1
        pszp = stack.enter_context(tc.tile_pool(name="pszp", bufs=1, space="PSUM"))
        ps1p = stack.enter_context(tc.tile_pool(name="ps1p", bufs=2, space="PSUM"))
        ps2p = stack.enter_context(tc.tile_pool(name="ps2p", bufs=2, space="PSUM"))

        chunk_state = {}

        def tree_a(eng, t):
            # sum taps 0..15 into t[:,0,:]
            for (a, b, n) in [(0, 8, 8), (0, 4, 4), (0, 2, 2), (0, 1, 1)]:
                eng.tensor_tensor(out=t[:, a:a + n, :], in0=t[:, a:a + n, :],
                                  in1=t[:, b:b + n, :], op=Alu.add)

        def phase1(ci):
            (r0c, nr) = CHUNKS[ci]
            nqc = nr * W
            vc = K2 * nqc
            feat = featp.tile([66, K2, nqc], f16, tag="feat", name=f"feat{ci}")
            nc.sync.dma_start(out=feat[64:66, :, :],
                                in_=rsubp_d[:, :, r0c * W:r0c * W + nqc])

            fv = feat[0:64]
            x1v = x1s[:, r0c:r0c + nr, :]
            # window subtracts: dj{0,2,4,6} and dj5 on DVE, dj{1,3} on GPSIMD
            for di in range(K):
                for (eng, off, ndj) in ((nc.vector, 0, 4),
                                        (nc.vector, 5, 1),
                                        (nc.gpsimd, 1, 2)):
                    x2w = win_ap(x2p[:], (r0c + di) * WP + off, ndj, nr)
                    x1w = bass.AP(tensor=x1v.tensor, offset=x1v.offset,
                                  ap=[x1v.ap[0], [0, ndj], x1v.ap[1], x1v.ap[2]])
                    outw = kq_ap(fv, di * K + off, ndj, nqc, nr)
                    eng.tensor_tensor(out=outw, in0=x1w, in1=x2w,
                                      op=Alu.subtract)
            for bq in range(4):
                ks = (K2 * bq) // 4, (K2 * (bq + 1)) // 4
                nc.vector.tensor_scalar_max(
                    out=feat[0:64, ks[0]:ks[1], :].rearrange("p a b -> p (a b)"),
                    in0=feat[0:64, ks[0]:ks[1], :].rearrange("p a b -> p (a b)"),
                    scalar1=0.0)

            featf = feat[:].rearrange("p a b -> p (a b)")
            # mm1 packs two 512-col blocks onto the 128 PSUM partitions so
            # the relu eviction covers half the free size
            h2w = (vc // 1024) * 512 + min(512, vc % 1024)
            h2 = h2p.tile([128, h2w], f16, tag="h2")
            for j0 in range(0, vc, 1024):
                n = min(1024, vc - j0)
                ps1 = ps1p.tile([128, 512], f32, tag="ps1")
                n0 = min(512, n)
                nc.tensor.matmul(ps1[0:64, :n0], cw1s,
                                 featf[:, j0:j0 + n0],
                                 start=True, stop=True)
                if n > 512:
                    nc.tensor.matmul(ps1[64:128, :n - 512], cw1s,
                                     featf[:, j0 + 512:j0 + n],
                                     start=True, stop=True)
                    nc.scalar.activation(
                        out=h2[:, j0 // 2:j0 // 2 + n - 512],
                        in_=ps1[:, :n - 512],
                        func=Act.Relu, bias=b2f, scale=1.0)
                    if n - 512 < 512:
                        nc.scalar.activation(
                            out=h2[0:64, j0 // 2 + n - 512:j0 // 2 + 512],
                            in_=ps1[0:64, n - 512:512],
                            func=Act.Relu, bias=b2f[0:64], scale=1.0)
                else:
                    nc.scalar.activation(
                        out=h2[0:64, j0 // 2:j0 // 2 + n],
                        in_=ps1[0:64, :n],
                        func=Act.Relu, bias=b2f[0:64], scale=1.0)

            e4 = e4p.tile([128, K2, nqc], f16, tag="e4")
            e4f = e4[:].rearrange("p a b -> p (a b)")
            for j0 in range(0, vc, 1024):
                n = min(1024, vc - j0)
                ps2 = ps2p.tile([128, 1024], f32, tag="ps2")
                for s in range(0, n, 512):
                    sn = min(512, n - s)
                    half = (s // 512) % 2
                    nc.tensor.matmul(
                        ps2[:, s:s + sn], cw2v(half),
                        h2[64 * half:64 * half + 64,
                           j0 // 2 + s - 512 * half:j0 // 2 + s - 512 * half + sn],
                        start=True, stop=True)
                nc.scalar.activation(out=e4f[:, j0:j0 + n], in_=ps2[:, :n],
                                     func=Act.Exp, bias=cb2r, scale=1.0)

            chunk_state[ci] = (e4,)

        def phase2(ci):
            (r0c, nr) = CHUNKS[ci]
            nqc = nr * W
            (e4,) = chunk_state[ci]
            # softmax normalizer on PE: Z = sum_k e4[:,k,:] via 49
            # identity-stationary matmuls accumulating in PSUM
            psz = pszp.tile([128, 768], f32, tag="psz", name=f"psz{ci}")
            for k in range(K2):
                nc.tensor.matmul(psz[:, 0:nqc], idv, e4[:, k, :],
                                 start=(k == 0), stop=(k == K2 - 1))
            prods = []
            for ot in range(2):
                prodt = prodp.tile([128, K2, nqc], f16, tag=f"prod{ot}",
                                   name=f"prod{ot}")
                prods.append(prodt)
                for di in range(K):
                    for par in range(2):
                        ndj = 4 if par == 0 else 3
                        sv = x3ps[:, ot, :]
                        k0 = di * K + par
                        x3w = bass.AP(
                            tensor=sv.tensor,
                            offset=sv.offset + (r0c + di) * WP + par,
                            ap=[sv.ap[0], [2, ndj], [WP, nr], [1, W]])
                        e4w = kq_ap(e4[:], k0, ndj, nqc, nr)
                        outw = kq_ap(prods[ot][:], k0, ndj, nqc, nr)
                        nc.vector.tensor_tensor(out=outw, in0=e4w, in1=x3w,
                                                op=Alu.mult)
            # taps 12..48 summed on PE into the agg accumulators; taps 0..11
            # tree-summed on GPSIMD
            for ot in range(2):
                a0 = 256 + 256 * ot
                for k in range(16, K2):
                    nc.tensor.matmul(psz[:, a0:a0 + nqc], idv,
                                     prods[ot][:, k, :],
                                     start=(k == 16), stop=(k == K2 - 1))
                tree_a(nc.gpsimd, prods[ot])

            rz = smallp.tile([128, nqc], f32, tag="rz")
            nc.vector.reciprocal(out=rz[:], in_=psz[:, 0:nqc])

            outb = []
            for ot in range(2):
                a0 = 256 + 256 * ot
                ob = smallp.tile([128, nqc], f32, tag=f"ob{ot}", name=f"ob{ot}")
                ob2 = smallp.tile([128, nqc], bf16, tag=f"ob2{ot}", name=f"ob2{ot}")
                outb.append(ob2)
                nc.vector.tensor_tensor(out=ob[:], in0=psz[:, a0:a0 + nqc],
                                        in1=prods[ot][:, 0, :], op=Alu.add)
                nc.vector.scalar_tensor_tensor(
                    out=ob[:], in0=ob[:], scalar=1.0, in1=rz[:],
                    op0=Alu.mult, op1=Alu.mult)
                nc.scalar.activation(out=ob2[:], in_=ob[:], func=Act.Relu,
                                     bias=b3fp[ot], scale=a3p[ot])

            for oo in range(2):
                psw = ps2p.tile([128, 1024], f32, tag="ps2")
                for kt in range(2):
                    nc.tensor.matmul(psw[:, :nqc], wcv(kt, oo),
                                     outb[kt][:],
                                     start=(kt == 0), stop=False)
                xi = xps[oo][:, PAD + r0c:PAD + r0c + nr, PAD:PAD + W]
                nc.tensor.matmul(psw[:, :nqc], idv,
                                 xi, start=False, stop=True)
                ysb = smallp.tile([128, nqc], f32, tag=f"ysb{oo}", name=f"ysb{oo}")
                nc.scalar.activation(out=ysb[:], in_=psw[:, :nqc],
                                     func=Act.Identity, bias=bcb[oo], scale=1.0)
                nc.sync.dma_start(
                    out=y_d[oo][:, r0c:r0c + nr, :],
                    in_=ysb[:].rearrange("p (r w) -> p r w", w=W))

        phase1(0)
        for ci in range(1, len(CHUNKS)):
            phase1(ci)
            phase2(ci - 1)
        phase2(len(CHUNKS) - 1)

    nc.compile()
    _BUILD_CACHE["nc"] = nc
    return nc


def _host_prep(inputs):
    f = {k: np.asarray(v, np.float32) for k, v in inputs.items()}

    def fold(n):
        a = f[n + "_g"] / np.sqrt(f[n + "_rv"] + EPS)
        return a, f[n + "_b"] - f[n + "_rm"] * a

    a1, b1f = fold("bn1")
    ac, bc1 = fold("cwbn1")
    a2, b2f = fold("cwbn2")
    a3, b3f = fold("bn2")

    W1p = ac[:64, None] * f["w1"]
    b1p = ac[:64] * f["b1"] + bc1[:64]
    W2p = ac[:64, None] * f["w2"]
    b2p = ac[:64] * f["b2"]
    cw1p = a2[:, None] * f["cw1"]

    perm = _perm_channels()
    w3p = f["w3"][perm]
    a3p = a3[perm]
    b3fp = b3f[perm]
    rep = np.arange(128) // 4
    cw2r = f["cw2"][rep]
    cb2r = f["cb2"][rep]

    locw = np.tile(np.linspace(-1.0, 1.0, W, dtype=np.float32)[None, :], (H, 1))
    loch = np.tile(np.linspace(-1.0, 1.0, H, dtype=np.float32)[:, None], (1, W))
    loc = np.stack([locw, loch], 0)
    p = np.einsum("chw,oc->ohw", loc, f["pw"]) + f["pb"][:, None, None]
    pp = np.pad(p, ((0, 0), (PAD, PAD), (PAD, PAD)), mode="reflect")
    pu = np.stack([pp[:, i:i + H, j:j + W] for i in range(K) for j in range(K)], 1)
    subp = p[:, None] - pu
    rsubp = np.maximum(ac[64:66, None, None, None] * subp
                       + bc1[64:66, None, None, None], 0).astype(np.float16)

    xpad = np.pad(f["x"], ((0, 0), (0, 0), (PAD, PAD), (PAD, PAD)), mode="reflect")

    w1T = np.ascontiguousarray(W1p.T).reshape(2, 128, 64)
    w2T = np.ascontiguousarray(W2p.T).reshape(2, 128, 64)
    wc_perm = f["wc"][:, perm]

    wb = np.zeros((128, WB_N), np.float32)
    for kt in range(2):
        wb[:, WB_W1 + 64 * kt:WB_W1 + 64 * (kt + 1)] = w1T[kt]
        wb[:, WB_W2 + 64 * kt:WB_W2 + 64 * (kt + 1)] = w2T[kt]
        for ot in range(2):
            wb[:, WB_W3 + 256 * kt + 128 * ot:WB_W3 + 256 * kt + 128 * (ot + 1)] = \
                w3p[ot * 128:(ot + 1) * 128, kt * 128:(kt + 1) * 128].T
            wb[:, WB_WC + 256 * kt + 128 * ot:WB_WC + 256 * kt + 128 * (ot + 1)] = \
                wc_perm[ot * 128:(ot + 1) * 128, kt * 128:(kt + 1) * 128].T
    wb[:, WB_ID:WB_ID + 128] = np.eye(128, dtype=np.float32)

    fbb = np.zeros((128, FB_N), np.float32)
    fbb[0:66, FB_CW1:FB_CW1 + 64] = cw1p.T
    fbb[0:64, FB_CW2:FB_CW2 + 128] = cw2r.T
    fbb[64:128, FB_CW2:FB_CW2 + 128] = cw2r.T

    scal = np.zeros((128, 14), np.float32)
    scal[:, 0] = a1[:128]; scal[:, 1] = a1[128:]
    scal[:, 2] = b1f[:128]; scal[:, 3] = b1f[128:]
    scal[:64, 4] = b1p; scal[:64, 5] = b2p
    scal[:64, 6] = b2f; scal[64:, 6] = b2f
    scal[:, 7] = cb2r
    scal[:, 8] = a3p[:128]; scal[:, 9] = a3p[128:]
    scal[:, 10] = b3fp[:128]; scal[:, 11] = b3fp[128:]
    scal[:, 12] = f["bc"][:128]; scal[:, 13] = f["bc"][128:]

    shared = dict(wb=wb.astype(bf16_np), fb=fbb.astype(np.float16), scal=scal)
    in_maps = []
    for core in range(8):
        b, i = divmod(core, 4)
        r0 = RB * i
        m = dict(shared)
        xc = xpad[b].reshape(2, 128, H + 2 * PAD, WP)[:, :, r0:r0 + ROWS, :]
        m["xp"] = np.ascontiguousarray(
            xc.transpose(1, 0, 2, 3)).astype(bf16_np)
        m["rsubp"] = np.ascontiguousarray(
            rsubp[:, :, r0:r0 + RB, :].reshape(2, K2, NQ))
        in_maps.append(m)
    return in_maps


def kernel(**inputs):
    from concourse.bass_utils import run_bass_kernel_spmd
    nc = _build_program()
    in_maps = _host_prep(inputs)
    res = run_bass_kernel_spmd(nc, in_maps, core_ids=list(range(8)))
    global LAST_RESULTS
    LAST_RESULTS = res
    y = np.zeros((B, C, H, W), np.float32)
    for core in range(8):
        b, i = divmod(core, 4)
        yc = res.results[core]["y"]
        y[b, :, RB * i:RB * (i + 1), :] = yc.reshape(C, RB, W)
    return y
